# revision 24
# baseline (speedup 1.0000x reference)
"""Multi-head attention (B=2, S=2048, D=1024, H=16) on 8 Trainium2 cores.

Sharding: core c handles batch c//4 and head-group c%4 (4 heads x dk 64).
Q/K/V projection weights are column-split by head group on the host; the
output projection is split by OUTPUT column: core c computes all 2048
tokens x its 256 output columns from the full gathered concat, so no
final collective or dynamic slice is needed.

Attention runs in 2 q-blocks of 1024 x 4 heads.  Scores stay in [k, q]
orientation; the PV product streams exp-scores against a stationary V
slice plus a ones column, giving [dk+1, q] with the softmax denominator
in row dk.  Normalization: rank-1 PE matmul broadcasts the denominator
row (keeping gpsimd free for collectives) -> DVE fast reciprocal ->
fused multiply.  Scores of block i+1 interleave with the PV matmuls of
block i so PE and ACT stay busy together.

Per-512-token-chunk AllGathers fire from the gpsimd queue as soon as a
chunk's heads are normalized, overlapping the remaining attention; the
column-split output projections all run after attention, where the ACT
engine is quiet and the PE is unthrottled, by which time only the last
chunk's gather is still in flight.
"""

import numpy as np
import ml_dtypes

import concourse.bass as bass
import concourse.tile as tile
from concourse import bacc, mybir
from concourse.bass_utils import run_bass_kernel_spmd

BF16 = mybir.dt.bfloat16
F32 = mybir.dt.float32
NPBF16 = ml_dtypes.bfloat16

B, S, D, H = 2, 2048, 1024, 16
DK = 64
DK1 = DK + 1
N_CORES = 8
HPC = 4               # heads per core
FEAT = HPC * DK       # 256 projected features per core
VW = HPC * DK1        # 260: v with a ones column per head
OCOL = 256            # output columns per core
TOKC = 1024           # token chunk for projections
QCH = 512             # q chunk for attention (= AllGather granularity)
NCH = S // QCH        # 4 chunks
NKT = S // 128        # 16 k tiles
NKC = D // 128        # 8 contraction chunks

_CACHE = {}


def _build_program():
    if "nc" in _CACHE:
        return _CACHE["nc"]

    nc = bacc.Bacc("TRN2", target_bir_lowering=False, debug=False,
                   num_devices=N_CORES)

    xq = nc.declare_dram_parameter("xq", [D, S], BF16, isOutput=False)
    xk = nc.declare_dram_parameter("xk", [D, S], BF16, isOutput=False)
    xv = nc.declare_dram_parameter("xv", [D, S], BF16, isOutput=False)
    wq = nc.declare_dram_parameter("wq", [D, FEAT], BF16, isOutput=False)
    wk = nc.declare_dram_parameter("wk", [D, FEAT], BF16, isOutput=False)
    wv = nc.declare_dram_parameter("wv", [D, VW], BF16, isOutput=False)
    wo = nc.declare_dram_parameter("wo", [D, OCOL], BF16, isOutput=False)
    bq = nc.declare_dram_parameter("bq", [128, 2], F32, isOutput=False)
    bk = nc.declare_dram_parameter("bk", [128, 2], F32, isOutput=False)
    bv = nc.declare_dram_parameter("bv", [1, VW], BF16, isOutput=False)
    bo = nc.declare_dram_parameter("bo", [1, OCOL], BF16, isOutput=False)
    out = nc.declare_dram_parameter("out", [S, OCOL], BF16, isOutput=True)
    dbg = {}

    with tile.TileContext(nc) as tc:
        with (
            tc.tile_pool(name="w", bufs=1) as wpool,
            tc.tile_pool(name="x", bufs=20) as xpool,
            tc.tile_pool(name="qk", bufs=1) as qkpool,
            tc.tile_pool(name="vp", bufs=1) as vpool,
            tc.tile_pool(name="sct", bufs=18) as sctpool,
            tc.tile_pool(name="nm", bufs=2) as nmpool,
            tc.tile_pool(name="cat", bufs=8) as catpool,
            tc.tile_pool(name="fo", bufs=3) as fopool,
            tc.tile_pool(name="ps_a", bufs=3, space="PSUM") as ps_a,
            tc.tile_pool(name="ps_pv", bufs=1, space="PSUM") as ps_pv,
            tc.tile_pool(name="dram", bufs=1, space="DRAM") as dram,
        ):
            _emit(nc, wpool, xpool, qkpool, vpool, sctpool, nmpool,
                  catpool, fopool, ps_a, ps_pv, dram,
                  xq, xk, xv, wq, wk, wv, wo, bq, bk, bv, bo, out, dbg)

    nc.compile()
    _CACHE["nc"] = nc
    return nc


def _emit(nc, wpool, xpool, qkpool, vpool, sctpool, nmpool, catpool,
          fopool, ps_a, ps_pv, dram,
          xq, xk, xv, wq, wk, wv, wo, bq, bk, bv, bo, out, dbg={}):
    MUL = mybir.AluOpType.mult
    EXPF = mybir.ActivationFunctionType.Exp
    IDF = mybir.ActivationFunctionType.Identity

    ones1 = wpool.tile([1, 128], BF16, tag="ones")
    nc.vector.memset(ones1[:], 1.0)

    # DMA issue engines for bulk input loads (round-robin: the Sync engine
    # alone issues descriptors at ~600ns each, which gates phase 1).  Only
    # SP and Activation are hardware-DGE engines; gpsimd DMA goes through
    # the software-DGE ring and corrupts data in this flow.
    dmae = [nc.sync, nc.scalar]
    NE = len(dmae)

    def load_x(src_t):
        # one 512 KB descriptor per contraction chunk: [128 rows, 4 KB/row]
        tiles = []
        for kc in range(NKC):
            t = xpool.tile([128, S], BF16, tag="xt")
            dmae[kc % NE].dma_start(t[:], src_t[bass.ts(kc, 128), :])
            tiles.append(t)
        return tiles

    wk_sb, xk_ts = [], []
    for kc in range(NKC):
        t = wpool.tile([128, FEAT], BF16, tag=f"wk{kc}")
        dmae[kc % NE].dma_start(t[:], wk[bass.ts(kc, 128), :])
        wk_sb.append(t)
        tx = xpool.tile([128, S], BF16, tag="xt")
        dmae[(kc + 1) % NE].dma_start(tx[:], xk[bass.ts(kc, 128), :])
        xk_ts.append(tx)
    bk_sb = wpool.tile([128, 2], F32, tag="bk")
    nc.sync.dma_start(bk_sb[:], bk[:])

    qh_sb = [qkpool.tile([128, S], BF16, tag=f"qh{m}", name=f"qh{m}")
             for m in range(2)]
    kh_sb = [qkpool.tile([128, S], BF16, tag=f"kh{m}", name=f"kh{m}")
             for m in range(2)]
    v_sb = [vpool.tile([128, VW], BF16, tag=f"v{j}", name=f"v{j}")
            for j in range(NKT)]

    def qk_group(w_sb, x_t, b_sb, dst, t0, m):
        ps = ps_a.tile([128, TOKC], F32, tag="a")
        for kc in range(NKC):
            for u in range(TOKC // 512):
                nc.tensor.matmul(
                    ps[:, bass.ts(u, 512)],
                    w_sb[kc][:, bass.ts(m, 128)],
                    x_t[kc][:, t0 * TOKC + u * 512:t0 * TOKC + (u + 1) * 512],
                    start=(kc == 0), stop=(kc == NKC - 1),
                )
        nc.vector.tensor_scalar_add(dst[m][:, bass.ts(t0, TOKC)], ps[:],
                                    b_sb[:, m:m + 1])

    # ---- K projection (scores need the full kh) ------------------
    for t0 in range(S // TOKC):
        for m in range(2):
            qk_group(wk_sb, xk_ts, bk_sb, kh_sb, t0, m)

    wq_sb = []
    for kc in range(NKC):
        t = wpool.tile([128, FEAT], BF16, tag=f"wq{kc}")
        dmae[kc % NE].dma_start(t[:], wq[bass.ts(kc, 128), :])
        wq_sb.append(t)
    bq_sb = wpool.tile([128, 2], F32, tag="bq")
    nc.sync.dma_start(bq_sb[:], bq[:])
    xq_ts = load_x(xq)
    for t0 in range(S // TOKC):
        for m in range(2):
            qk_group(wq_sb, xq_ts, bq_sb, qh_sb, t0, m)

    # ---- V weights + inputs --------------------------------------
    wv_sb = []
    for kc in range(NKC):
        t = wpool.tile([128, VW], BF16, tag=f"wv{kc}")
        dmae[kc % NE].dma_start(t[:], wv[bass.ts(kc, 128), :])
        wv_sb.append(t)
    bv_sb = wpool.tile([1, VW], BF16, tag="bv")
    nc.sync.dma_start(bv_sb[:], bv[:])
    xv_ts = load_x(xv)

    def v_group(t0, j):
        ps = ps_a.tile([128, VW], F32, tag="a")
        for kc in range(NKC):
            nc.tensor.matmul(
                ps[:],
                xv_ts[kc][:, t0 * TOKC + j * 128:t0 * TOKC + (j + 1) * 128],
                wv_sb[kc][:],
                start=(kc == 0), stop=False,
            )
        nc.tensor.matmul(ps[:], ones1[:], bv_sb[:], start=False, stop=True)
        nc.vector.tensor_copy(v_sb[t0 * (TOKC // 128) + j][:], ps[:])

    for t0 in range(S // TOKC):
        for j in range(TOKC // 128):
            v_group(t0, j)

    # wo + bo requested now: the 0.5 MB load drains during attention.
    wo_sb = []
    for kc in range(NKC):
        t = wpool.tile([128, OCOL], BF16, tag=f"wo{kc}")
        dmae[kc % NE].dma_start(t[:], wo[bass.ts(kc, 128), :])
        wo_sb.append(t)
    bo_sb = wpool.tile([1, OCOL], BF16, tag="bo")
    nc.sync.dma_start(bo_sb[:], bo[:])

    # ---- phase 2/3: attention + chunked AllGather + out proj -----
    # Attention runs in 2 q-blocks of 1024 x 4 heads (the baseline shape,
    # which paces best under the power throttle).  AllGathers fire per
    # 512-token chunk as soon as its heads are normalized; chunk 3 ships
    # in two half-gathers (heads 01 early, heads 23 at the end).  The
    # column-split output projections all run after attention, where the
    # ACT engine is quiet and the PE runs unthrottled; their collectives
    # are complete by then, so there is no dead zone.
    QB = 2 * QCH          # 1024-token attention block
    NQB = S // QB         # 2 blocks
    # Collectives cost ~15-20us nearly independent of size, so pack them
    # to keep the CC stream clear for the final gather: q-block 0 ships as
    # ONE 4-head gather; q-block 1 ships heads 0 and 1 individually (the
    # stream is idle then) and heads 2+3 as a single pair-gather, which is
    # the only collective left on the critical path.
    agm_in = dram.tile([FEAT, QB], BF16, tag="agmi", name="agmi")
    agm_out = dram.tile([4 * FEAT, QB], BF16, tag="agmo", name="agmo")
    agh_in = [dram.tile([DK, QB], BF16, tag=f"aghi{h}", name=f"aghi{h}")
              for h in range(2)]
    agh_out = [dram.tile([4 * DK, QB], BF16, tag=f"agho{h}",
                         name=f"agho{h}") for h in range(2)]
    agp_in = dram.tile([2 * DK, QB], BF16, tag="agpi", name="agpi")
    agp_out = dram.tile([4 * 2 * DK, QB], BF16, tag="agpo", name="agpo")

    def emit_ag(eng, ins_ap, outs_ap):
        eng.collective_compute(
            "AllGather", mybir.AluOpType.bypass,
            replica_groups=[[0, 1, 2, 3], [4, 5, 6, 7]],
            ins=[ins_ap.opt()],
            outs=[outs_ap.opt()],
        )

    def norm_and_out(pv, h, qb):
        pvs = nmpool.tile([DK1, QB], F32, tag="pvs")
        nc.vector.tensor_copy(pvs[:], pv[:])
        drow = nmpool.tile([1, QB], BF16, tag="drow")
        nc.vector.tensor_copy(drow[:], pvs[DK:DK1, :])
        # Broadcast the denominator row via a rank-1 PE matmul instead of
        # gpsimd partition_broadcast, keeping the gpsimd queue free for the
        # AllGathers (its queue blocks while a collective is in flight).
        psb = ps_a.tile([DK, QB], F32, tag="a", name="psb")
        for u in range(QB // 512):
            nc.tensor.matmul(psb[:, bass.ts(u, 512)], ones1[0:1, 0:DK],
                             drow[:, bass.ts(u, 512)], start=True, stop=True)
        dbs = nmpool.tile([DK, QB], F32, tag="db")
        nc.vector.tensor_copy(dbs[:], psb[:])
        rb = nmpool.tile([DK, QB], F32, tag="rb")
        nc.vector.reciprocal_approx_fast(rb[:], dbs[:])
        onrm = nmpool.tile([DK, QB], BF16, tag="onrm")
        nc.vector.scalar_tensor_tensor(onrm[:], pvs[0:DK, :], 1.0, rb[:],
                                       MUL, MUL)
        if qb == 0:
            nc.sync.dma_start(agm_in[h * DK:(h + 1) * DK, :], onrm[:])
            if h == HPC - 1:
                emit_ag(nc.gpsimd, agm_in[:], agm_out[:])
        elif h < 2:
            nc.sync.dma_start(agh_in[h][:], onrm[:])
            emit_ag(nc.gpsimd, agh_in[h][:], agh_out[h][:])
        else:
            nc.sync.dma_start(agp_in[(h - 2) * DK:(h - 1) * DK, :], onrm[:])
            if h == HPC - 1:
                emit_ag(nc.gpsimd, agp_in[:], agp_out[:])

    def out_proj(c):
        qb, u = c // 2, c % 2
        # For q-block 1 the heads-0/1 features arrive (agh gathers) well
        # before the heads-2/3 pair gather: load and accumulate them first
        # so half of each tail GEMM overlaps the final collective.
        kc_order = list(range(NKC)) if qb == 0 else [0, 1, 4, 5, 2, 3, 6, 7]
        cat = {}
        for kc in kc_order:
            # features kc*128 = core r = kc//2, heads 2*(kc%2), 2*(kc%2)+1
            r, hp = kc // 2, 2 * (kc % 2)
            t = catpool.tile([128, QCH], BF16, tag="cat")
            if qb == 0:
                dmae[kc % NE].dma_start(
                    t[:], agm_out[bass.ts(kc, 128), bass.ts(u, QCH)])
            elif hp == 0:
                for hh in range(2):
                    dmae[(kc + hh) % NE].dma_start(
                        t[hh * DK:(hh + 1) * DK, :],
                        agh_out[hh][bass.ts(r, DK), bass.ts(u, QCH)])
            else:
                dmae[kc % NE].dma_start(
                    t[:], agp_out[bass.ts(r, 128), bass.ts(u, QCH)])
            cat[kc] = t
        for qt in range(QCH // 128):
            ps = ps_a.tile([128, OCOL], F32, tag="a", name="po")
            nc.tensor.matmul(ps[:], ones1[:, 0:128], bo_sb[:],
                             start=True, stop=False)
            for i, kc in enumerate(kc_order):
                nc.tensor.matmul(
                    ps[:],
                    cat[kc][:, bass.ts(qt, 128)],
                    wo_sb[kc][:],
                    start=False, stop=(i == NKC - 1),
                )
            fo = fopool.tile([128, OCOL], BF16, tag="fo")
            nc.scalar.activation(fo[:], ps[:], IDF)
            dmae[qt % NE].dma_start(
                out[bass.ts(c * (QCH // 128) + qt, 128), :], fo[:])

    blocks = [(qb, h) for qb in range(NQB) for h in range(HPC)]
    last = len(blocks) - 1
    prev = None
    for bi, (qb, h) in enumerate(blocks):
        ht, hr = h // 2, (h % 2) * 64
        q0 = qb * QB
        if bi == last:
            pv = ps_a.tile([DK1, QB], F32, tag="a", name="pv_last")
        else:
            pv = ps_pv.tile([DK1, QB], F32, tag="pv")
        cur_sc = []
        for kt in range(NKT):
            ps = ps_a.tile([128, QB], F32, tag="a")
            for u in range(QB // 512):
                nc.tensor.matmul(
                    ps[:, bass.ts(u, 512)],
                    kh_sb[ht][hr:hr + 64, bass.ts(kt, 128)],
                    qh_sb[ht][hr:hr + 64, q0 + u * 512:q0 + (u + 1) * 512],
                    start=True, stop=True,
                )
            sct = sctpool.tile([128, QB], BF16, tag="sct", name="sct")
            nc.scalar.activation(sct[:], ps[:], EXPF, scale=0.125)
            cur_sc.append(sct)
            if prev is not None:
                ppv, psc, ph, pqb = prev
                for u in range(QB // 512):
                    nc.tensor.matmul(
                        ppv[:, bass.ts(u, 512)],
                        v_sb[kt][:, ph * DK1:(ph + 1) * DK1],
                        psc[kt][:, bass.ts(u, 512)],
                        start=(kt == 0), stop=(kt == NKT - 1),
                    )
            if bi == last and kt >= 1:
                # self-interleave: the last block folds its own PV in with
                # a one-slot lag so the drain after the loop is only kt=15.
                for u in range(QB // 512):
                    nc.tensor.matmul(
                        pv[:, bass.ts(u, 512)],
                        v_sb[kt - 1][:, h * DK1:(h + 1) * DK1],
                        cur_sc[kt - 1][:, bass.ts(u, 512)],
                        start=(kt - 1 == 0), stop=False,
                    )
        if prev is not None:
            norm_and_out(prev[0], prev[2], prev[3])
        prev = (pv, cur_sc, h, qb)

    # drain: the last block only needs kt=15
    ppv, psc, ph, pqb = prev
    for u in range(QB // 512):
        nc.tensor.matmul(
            ppv[:, bass.ts(u, 512)],
            v_sb[NKT - 1][:, ph * DK1:(ph + 1) * DK1],
            psc[NKT - 1][:, bass.ts(u, 512)],
            start=False, stop=True,
        )
    norm_and_out(ppv, ph, pqb)
    for c in range(NCH):
        out_proj(c)


def _prep_inputs(q, k, v, Wq, bq, Wk, bk, Wv, bv, Wo, bo):
    """Build the per-core input maps (host-side sharding)."""
    in_maps = []
    for c in range(N_CORES):
        b, hg = c // 4, c % 4
        fsl = slice(FEAT * hg, FEAT * (hg + 1))
        osl = slice(OCOL * hg, OCOL * (hg + 1))
        wv_aug = np.zeros((D, VW), np.float32)
        bv_aug = np.zeros((VW,), np.float32)
        for h in range(HPC):
            rows = slice(FEAT * hg + DK * h, FEAT * hg + DK * (h + 1))
            wv_aug[:, h * DK1:h * DK1 + DK] = Wv[rows, :].T
            bv_aug[h * DK1:h * DK1 + DK] = bv[rows]
            bv_aug[h * DK1 + DK] = 1.0
        in_maps.append({
            "xq": np.ascontiguousarray(q[b].T).astype(NPBF16),
            "xk": np.ascontiguousarray(k[b].T).astype(NPBF16),
            "xv": np.ascontiguousarray(v[b].T).astype(NPBF16),
            "wq": np.ascontiguousarray(Wq[fsl].T).astype(NPBF16),
            "wk": np.ascontiguousarray(Wk[fsl].T).astype(NPBF16),
            "wv": wv_aug.astype(NPBF16),
            "wo": np.ascontiguousarray(Wo[osl].T).astype(NPBF16),
            "bq": np.ascontiguousarray(
                bq[fsl].reshape(2, 128).T).astype(np.float32),
            "bk": np.ascontiguousarray(
                bk[fsl].reshape(2, 128).T).astype(np.float32),
            "bv": bv_aug.reshape(1, VW).astype(NPBF16),
            "bo": np.ascontiguousarray(
                bo[osl].reshape(1, OCOL)).astype(NPBF16),
        })
    return in_maps


def run_sharded(in_maps, trace=False):
    nc = _build_program()
    res = run_bass_kernel_spmd(nc, in_maps, list(range(N_CORES)), trace=trace)
    full = np.empty((B, S, D), np.float32)
    for c in range(N_CORES):
        b, hg = c // 4, c % 4
        full[b, :, OCOL * hg:OCOL * (hg + 1)] = (
            res.results[c]["out"].astype(np.float32))
    return full, res


def kernel(q, k, v, Wq, bq, Wk, bk, Wv, bv, Wo, bo):
    args = [np.asarray(x, np.float32) for x in
            (q, k, v, Wq, bq, Wk, bk, Wv, bv, Wo, bo)]
    in_maps = _prep_inputs(*args)
    full, _ = run_sharded(in_maps)
    return full


# revision 25
# speedup vs baseline: 1.0146x; 1.0146x over previous
"""Multi-head attention (B=2, S=2048, D=1024, H=16) on 8 Trainium2 cores.

Sharding: core c handles batch c//4 and head-group c%4 (4 heads x dk 64).
Q/K/V projection weights are column-split by head group on the host; the
output projection is split by OUTPUT column: core c computes all 2048
tokens x its 256 output columns from the full gathered concat, so no
final collective or dynamic slice is needed.

Phase 1 loads x with one 512 KB full-row descriptor per contraction
chunk, issue split across the two hardware-DGE engines (SP+ACT).
Attention runs in 2 q-blocks of 1024 x 4 heads, ACT(exp)-paced; scores
stay in [k, q] orientation and the PV product streams exp-scores against
a stationary V slice plus a ones column, giving [dk+1, q] with the
softmax denominator in row dk.  A rank-1 PE matmul broadcasts the
denominator row (keeping gpsimd free for collectives) -> DVE fast
reciprocal -> fused multiply.  Scores of block i+1 interleave with the
PV matmuls of block i.

Collectives cost ~15-20us nearly independent of size, so the gather
schedule packs them: q-block 0 ships as one 4-head AllGather mid-
attention; q-block 1 ships heads 0/1 individually while the CC stream is
idle and heads 2+3 as a single pair-gather - the only collective on the
critical path.  The column-split output projections run after attention
(ACT quiet, PE unthrottled); the two q-block-1 projections accumulate
their early-gathered features first so half of each tail GEMM overlaps
the final pair-gather.
"""

import numpy as np
import ml_dtypes

import concourse.bass as bass
import concourse.tile as tile
from concourse import bacc, mybir
from concourse.bass_utils import run_bass_kernel_spmd

BF16 = mybir.dt.bfloat16
F32 = mybir.dt.float32
NPBF16 = ml_dtypes.bfloat16

B, S, D, H = 2, 2048, 1024, 16
DK = 64
DK1 = DK + 1
N_CORES = 8
HPC = 4               # heads per core
FEAT = HPC * DK       # 256 projected features per core
VW = HPC * DK1        # 260: v with a ones column per head
OCOL = 256            # output columns per core
TOKC = 1024           # token chunk for projections
QCH = 512             # q chunk for attention (= AllGather granularity)
NCH = S // QCH        # 4 chunks
NKT = S // 128        # 16 k tiles
NKC = D // 128        # 8 contraction chunks

_CACHE = {}


def _build_program():
    if "nc" in _CACHE:
        return _CACHE["nc"]

    nc = bacc.Bacc("TRN2", target_bir_lowering=False, debug=False,
                   num_devices=N_CORES)

    xq = nc.declare_dram_parameter("xq", [D, S], BF16, isOutput=False)
    xk = nc.declare_dram_parameter("xk", [D, S], BF16, isOutput=False)
    xv = nc.declare_dram_parameter("xv", [D, S], BF16, isOutput=False)
    wq = nc.declare_dram_parameter("wq", [D, FEAT], BF16, isOutput=False)
    wk = nc.declare_dram_parameter("wk", [D, FEAT], BF16, isOutput=False)
    wv = nc.declare_dram_parameter("wv", [D, VW], BF16, isOutput=False)
    wo = nc.declare_dram_parameter("wo", [D, OCOL], BF16, isOutput=False)
    bq = nc.declare_dram_parameter("bq", [128, 2], F32, isOutput=False)
    bk = nc.declare_dram_parameter("bk", [128, 2], F32, isOutput=False)
    bv = nc.declare_dram_parameter("bv", [1, VW], BF16, isOutput=False)
    bo = nc.declare_dram_parameter("bo", [1, OCOL], BF16, isOutput=False)
    out = nc.declare_dram_parameter("out", [S, OCOL], BF16, isOutput=True)
    dbg = {}

    with tile.TileContext(nc) as tc:
        with (
            tc.tile_pool(name="w", bufs=1) as wpool,
            tc.tile_pool(name="x", bufs=20) as xpool,
            tc.tile_pool(name="qk", bufs=1) as qkpool,
            tc.tile_pool(name="vp", bufs=1) as vpool,
            tc.tile_pool(name="sct", bufs=18) as sctpool,
            tc.tile_pool(name="nm", bufs=2) as nmpool,
            tc.tile_pool(name="cat", bufs=8) as catpool,
            tc.tile_pool(name="fo", bufs=3) as fopool,
            tc.tile_pool(name="ps_a", bufs=3, space="PSUM") as ps_a,
            tc.tile_pool(name="ps_pv", bufs=1, space="PSUM") as ps_pv,
            tc.tile_pool(name="dram", bufs=1, space="DRAM") as dram,
        ):
            _emit(nc, wpool, xpool, qkpool, vpool, sctpool, nmpool,
                  catpool, fopool, ps_a, ps_pv, dram,
                  xq, xk, xv, wq, wk, wv, wo, bq, bk, bv, bo, out, dbg)

    nc.compile()
    _CACHE["nc"] = nc
    return nc


def _emit(nc, wpool, xpool, qkpool, vpool, sctpool, nmpool, catpool,
          fopool, ps_a, ps_pv, dram,
          xq, xk, xv, wq, wk, wv, wo, bq, bk, bv, bo, out, dbg={}):
    MUL = mybir.AluOpType.mult
    EXPF = mybir.ActivationFunctionType.Exp
    IDF = mybir.ActivationFunctionType.Identity

    ones1 = wpool.tile([1, 128], BF16, tag="ones")
    nc.vector.memset(ones1[:], 1.0)

    # DMA issue engines for bulk input loads (round-robin: the Sync engine
    # alone issues descriptors at ~600ns each, which gates phase 1).  Only
    # SP and Activation are hardware-DGE engines; gpsimd DMA goes through
    # the software-DGE ring and corrupts data in this flow.
    dmae = [nc.sync, nc.scalar]
    NE = len(dmae)

    def load_x(src_t):
        # one 512 KB descriptor per contraction chunk: [128 rows, 4 KB/row]
        tiles = []
        for kc in range(NKC):
            t = xpool.tile([128, S], BF16, tag="xt")
            dmae[kc % NE].dma_start(t[:], src_t[bass.ts(kc, 128), :])
            tiles.append(t)
        return tiles

    wk_sb, xk_ts = [], []
    for kc in range(NKC):
        t = wpool.tile([128, FEAT], BF16, tag=f"wk{kc}")
        dmae[kc % NE].dma_start(t[:], wk[bass.ts(kc, 128), :])
        wk_sb.append(t)
        tx = xpool.tile([128, S], BF16, tag="xt")
        dmae[(kc + 1) % NE].dma_start(tx[:], xk[bass.ts(kc, 128), :])
        xk_ts.append(tx)
    bk_sb = wpool.tile([128, 2], F32, tag="bk")
    nc.sync.dma_start(bk_sb[:], bk[:])

    qh_sb = [qkpool.tile([128, S], BF16, tag=f"qh{m}", name=f"qh{m}")
             for m in range(2)]
    kh_sb = [qkpool.tile([128, S], BF16, tag=f"kh{m}", name=f"kh{m}")
             for m in range(2)]
    v_sb = [vpool.tile([128, VW], BF16, tag=f"v{j}", name=f"v{j}")
            for j in range(NKT)]

    def qk_group(w_sb, x_t, b_sb, dst, t0, m):
        ps = ps_a.tile([128, TOKC], F32, tag="a")
        for kc in range(NKC):
            for u in range(TOKC // 512):
                nc.tensor.matmul(
                    ps[:, bass.ts(u, 512)],
                    w_sb[kc][:, bass.ts(m, 128)],
                    x_t[kc][:, t0 * TOKC + u * 512:t0 * TOKC + (u + 1) * 512],
                    start=(kc == 0), stop=(kc == NKC - 1),
                )
        nc.vector.tensor_scalar_add(dst[m][:, bass.ts(t0, TOKC)], ps[:],
                                    b_sb[:, m:m + 1])

    # ---- K projection (scores need the full kh) ------------------
    for t0 in range(S // TOKC):
        for m in range(2):
            qk_group(wk_sb, xk_ts, bk_sb, kh_sb, t0, m)

    wq_sb = []
    for kc in range(NKC):
        t = wpool.tile([128, FEAT], BF16, tag=f"wq{kc}")
        dmae[kc % NE].dma_start(t[:], wq[bass.ts(kc, 128), :])
        wq_sb.append(t)
    bq_sb = wpool.tile([128, 2], F32, tag="bq")
    nc.sync.dma_start(bq_sb[:], bq[:])
    xq_ts = load_x(xq)
    for t0 in range(S // TOKC):
        for m in range(2):
            qk_group(wq_sb, xq_ts, bq_sb, qh_sb, t0, m)

    # ---- V weights + inputs --------------------------------------
    wv_sb = []
    for kc in range(NKC):
        t = wpool.tile([128, VW], BF16, tag=f"wv{kc}")
        dmae[kc % NE].dma_start(t[:], wv[bass.ts(kc, 128), :])
        wv_sb.append(t)
    bv_sb = wpool.tile([1, VW], BF16, tag="bv")
    nc.sync.dma_start(bv_sb[:], bv[:])
    xv_ts = load_x(xv)

    def v_group(t0, j):
        ps = ps_a.tile([128, VW], F32, tag="a")
        for kc in range(NKC):
            nc.tensor.matmul(
                ps[:],
                xv_ts[kc][:, t0 * TOKC + j * 128:t0 * TOKC + (j + 1) * 128],
                wv_sb[kc][:],
                start=(kc == 0), stop=False,
            )
        nc.tensor.matmul(ps[:], ones1[:], bv_sb[:], start=False, stop=True)
        nc.vector.tensor_copy(v_sb[t0 * (TOKC // 128) + j][:], ps[:])

    for t0 in range(S // TOKC):
        for j in range(TOKC // 128):
            v_group(t0, j)

    # wo + bo requested now: the 0.5 MB load drains during attention.
    wo_sb = []
    for kc in range(NKC):
        t = wpool.tile([128, OCOL], BF16, tag=f"wo{kc}")
        dmae[kc % NE].dma_start(t[:], wo[bass.ts(kc, 128), :])
        wo_sb.append(t)
    bo_sb = wpool.tile([1, OCOL], BF16, tag="bo")
    nc.sync.dma_start(bo_sb[:], bo[:])

    # ---- phase 2/3: attention + chunked AllGather + out proj -----
    # Attention runs in 2 q-blocks of 1024 x 4 heads (the baseline shape,
    # which paces best under the power throttle).  AllGathers fire per
    # 512-token chunk as soon as its heads are normalized; chunk 3 ships
    # in two half-gathers (heads 01 early, heads 23 at the end).  The
    # column-split output projections all run after attention, where the
    # ACT engine is quiet and the PE runs unthrottled; their collectives
    # are complete by then, so there is no dead zone.
    QB = 2 * QCH          # 1024-token attention block
    NQB = S // QB         # 2 blocks
    # Collectives cost ~15-20us nearly independent of size, so pack them
    # to keep the CC stream clear for the final gather: q-block 0 ships as
    # ONE 4-head gather; q-block 1 ships heads 0 and 1 individually (the
    # stream is idle then) and heads 2+3 as a single pair-gather, which is
    # the only collective left on the critical path.
    agm_in = dram.tile([FEAT, QB], BF16, tag="agmi", name="agmi")
    agm_out = dram.tile([4 * FEAT, QB], BF16, tag="agmo", name="agmo")
    agh_in = [dram.tile([DK, QB], BF16, tag=f"aghi{h}", name=f"aghi{h}")
              for h in range(2)]
    agh_out = [dram.tile([4 * DK, QB], BF16, tag=f"agho{h}",
                         name=f"agho{h}") for h in range(2)]
    agp_in = dram.tile([2 * DK, QB], BF16, tag="agpi", name="agpi")
    agp_out = dram.tile([4 * 2 * DK, QB], BF16, tag="agpo", name="agpo")

    def emit_ag(eng, ins_ap, outs_ap):
        eng.collective_compute(
            "AllGather", mybir.AluOpType.bypass,
            replica_groups=[[0, 1, 2, 3], [4, 5, 6, 7]],
            ins=[ins_ap.opt()],
            outs=[outs_ap.opt()],
        )

    def norm_and_out(pv, h, qb):
        pvs = nmpool.tile([DK1, QB], F32, tag="pvs")
        nc.vector.tensor_copy(pvs[:], pv[:])
        drow = nmpool.tile([1, QB], BF16, tag="drow")
        nc.vector.tensor_copy(drow[:], pvs[DK:DK1, :])
        # Broadcast the denominator row via a rank-1 PE matmul instead of
        # gpsimd partition_broadcast, keeping the gpsimd queue free for the
        # AllGathers (its queue blocks while a collective is in flight).
        psb = ps_a.tile([DK, QB], F32, tag="a", name="psb")
        for u in range(QB // 512):
            nc.tensor.matmul(psb[:, bass.ts(u, 512)], ones1[0:1, 0:DK],
                             drow[:, bass.ts(u, 512)], start=True, stop=True)
        dbs = nmpool.tile([DK, QB], F32, tag="db")
        nc.vector.tensor_copy(dbs[:], psb[:])
        rb = nmpool.tile([DK, QB], F32, tag="rb")
        nc.vector.reciprocal_approx_fast(rb[:], dbs[:])
        onrm = nmpool.tile([DK, QB], BF16, tag="onrm")
        nc.vector.scalar_tensor_tensor(onrm[:], pvs[0:DK, :], 1.0, rb[:],
                                       MUL, MUL)
        if qb == 0:
            nc.sync.dma_start(agm_in[h * DK:(h + 1) * DK, :], onrm[:])
            if h == HPC - 1:
                emit_ag(nc.gpsimd, agm_in[:], agm_out[:])
        elif h < 2:
            nc.sync.dma_start(agh_in[h][:], onrm[:])
            emit_ag(nc.gpsimd, agh_in[h][:], agh_out[h][:])
        else:
            nc.sync.dma_start(agp_in[(h - 2) * DK:(h - 1) * DK, :], onrm[:])
            if h == HPC - 1:
                emit_ag(nc.gpsimd, agp_in[:], agp_out[:])

    def out_proj(c):
        qb, u = c // 2, c % 2
        # For q-block 1 the heads-0/1 features arrive (agh gathers) well
        # before the heads-2/3 pair gather: load and accumulate them first
        # so half of each tail GEMM overlaps the final collective.
        kc_order = list(range(NKC)) if qb == 0 else [0, 1, 4, 5, 2, 3, 6, 7]
        cat = {}
        for kc in kc_order:
            # features kc*128 = core r = kc//2, heads 2*(kc%2), 2*(kc%2)+1
            r, hp = kc // 2, 2 * (kc % 2)
            t = catpool.tile([128, QCH], BF16, tag="cat")
            if qb == 0:
                dmae[kc % NE].dma_start(
                    t[:], agm_out[bass.ts(kc, 128), bass.ts(u, QCH)])
            elif hp == 0:
                for hh in range(2):
                    dmae[(kc + hh) % NE].dma_start(
                        t[hh * DK:(hh + 1) * DK, :],
                        agh_out[hh][bass.ts(r, DK), bass.ts(u, QCH)])
            else:
                dmae[kc % NE].dma_start(
                    t[:], agp_out[bass.ts(r, 128), bass.ts(u, QCH)])
            cat[kc] = t
        for qt in range(QCH // 128):
            ps = ps_a.tile([128, OCOL], F32, tag="a", name="po")
            nc.tensor.matmul(ps[:], ones1[:, 0:128], bo_sb[:],
                             start=True, stop=False)
            for i, kc in enumerate(kc_order):
                nc.tensor.matmul(
                    ps[:],
                    cat[kc][:, bass.ts(qt, 128)],
                    wo_sb[kc][:],
                    start=False, stop=(i == NKC - 1),
                )
            fo = fopool.tile([128, OCOL], BF16, tag="fo")
            nc.scalar.activation(fo[:], ps[:], IDF)
            dmae[qt % NE].dma_start(
                out[bass.ts(c * (QCH // 128) + qt, 128), :], fo[:])

    blocks = [(qb, h) for qb in range(NQB) for h in range(HPC)]
    last = len(blocks) - 1
    prev = None
    for bi, (qb, h) in enumerate(blocks):
        ht, hr = h // 2, (h % 2) * 64
        q0 = qb * QB
        if bi == last:
            pv = ps_a.tile([DK1, QB], F32, tag="a", name="pv_last")
        else:
            pv = ps_pv.tile([DK1, QB], F32, tag="pv")
        cur_sc = []
        for kt in range(NKT):
            ps = ps_a.tile([128, QB], F32, tag="a")
            for u in range(QB // 512):
                nc.tensor.matmul(
                    ps[:, bass.ts(u, 512)],
                    kh_sb[ht][hr:hr + 64, bass.ts(kt, 128)],
                    qh_sb[ht][hr:hr + 64, q0 + u * 512:q0 + (u + 1) * 512],
                    start=True, stop=True,
                )
            sct = sctpool.tile([128, QB], BF16, tag="sct", name="sct")
            nc.scalar.activation(sct[:], ps[:], EXPF, scale=0.125)
            cur_sc.append(sct)
            if prev is not None:
                ppv, psc, ph, pqb = prev
                for u in range(QB // 512):
                    nc.tensor.matmul(
                        ppv[:, bass.ts(u, 512)],
                        v_sb[kt][:, ph * DK1:(ph + 1) * DK1],
                        psc[kt][:, bass.ts(u, 512)],
                        start=(kt == 0), stop=(kt == NKT - 1),
                    )
            if bi == last and kt >= 1:
                # self-interleave: the last block folds its own PV in with
                # a one-slot lag so the drain after the loop is only kt=15.
                for u in range(QB // 512):
                    nc.tensor.matmul(
                        pv[:, bass.ts(u, 512)],
                        v_sb[kt - 1][:, h * DK1:(h + 1) * DK1],
                        cur_sc[kt - 1][:, bass.ts(u, 512)],
                        start=(kt - 1 == 0), stop=False,
                    )
        if prev is not None:
            norm_and_out(prev[0], prev[2], prev[3])
        prev = (pv, cur_sc, h, qb)

    # drain: the last block only needs kt=15
    ppv, psc, ph, pqb = prev
    for u in range(QB // 512):
        nc.tensor.matmul(
            ppv[:, bass.ts(u, 512)],
            v_sb[NKT - 1][:, ph * DK1:(ph + 1) * DK1],
            psc[NKT - 1][:, bass.ts(u, 512)],
            start=False, stop=True,
        )
    norm_and_out(ppv, ph, pqb)
    for c in range(NCH):
        out_proj(c)


def _prep_inputs(q, k, v, Wq, bq, Wk, bk, Wv, bv, Wo, bo):
    """Build the per-core input maps (host-side sharding)."""
    in_maps = []
    for c in range(N_CORES):
        b, hg = c // 4, c % 4
        fsl = slice(FEAT * hg, FEAT * (hg + 1))
        osl = slice(OCOL * hg, OCOL * (hg + 1))
        wv_aug = np.zeros((D, VW), np.float32)
        bv_aug = np.zeros((VW,), np.float32)
        for h in range(HPC):
            rows = slice(FEAT * hg + DK * h, FEAT * hg + DK * (h + 1))
            wv_aug[:, h * DK1:h * DK1 + DK] = Wv[rows, :].T
            bv_aug[h * DK1:h * DK1 + DK] = bv[rows]
            bv_aug[h * DK1 + DK] = 1.0
        in_maps.append({
            "xq": np.ascontiguousarray(q[b].T).astype(NPBF16),
            "xk": np.ascontiguousarray(k[b].T).astype(NPBF16),
            "xv": np.ascontiguousarray(v[b].T).astype(NPBF16),
            "wq": np.ascontiguousarray(Wq[fsl].T).astype(NPBF16),
            "wk": np.ascontiguousarray(Wk[fsl].T).astype(NPBF16),
            "wv": wv_aug.astype(NPBF16),
            "wo": np.ascontiguousarray(Wo[osl].T).astype(NPBF16),
            "bq": np.ascontiguousarray(
                bq[fsl].reshape(2, 128).T).astype(np.float32),
            "bk": np.ascontiguousarray(
                bk[fsl].reshape(2, 128).T).astype(np.float32),
            "bv": bv_aug.reshape(1, VW).astype(NPBF16),
            "bo": np.ascontiguousarray(
                bo[osl].reshape(1, OCOL)).astype(NPBF16),
        })
    return in_maps


def run_sharded(in_maps, trace=False):
    nc = _build_program()
    res = run_bass_kernel_spmd(nc, in_maps, list(range(N_CORES)), trace=trace)
    full = np.empty((B, S, D), np.float32)
    for c in range(N_CORES):
        b, hg = c // 4, c % 4
        full[b, :, OCOL * hg:OCOL * (hg + 1)] = (
            res.results[c]["out"].astype(np.float32))
    return full, res


def kernel(q, k, v, Wq, bq, Wk, bk, Wv, bv, Wo, bo):
    args = [np.asarray(x, np.float32) for x in
            (q, k, v, Wq, bq, Wk, bk, Wv, bv, Wo, bo)]
    in_maps = _prep_inputs(*args)
    full, _ = run_sharded(in_maps)
    return full


# revision 27
# speedup vs baseline: 1.0653x; 1.0500x over previous
"""Multi-head attention (B=2, S=2048, D=1024, H=16) on 8 Trainium2 cores.

Sharding: core c handles batch c//4 and head-group c%4 (4 heads x dk 64).
Q/K/V projection weights are column-split by head group on the host; the
output projection is split by OUTPUT column: core c computes all 2048
tokens x its 256 output columns from the full gathered concat, so no
final collective or dynamic slice is needed.

Phase 1 loads x with one 512 KB full-row descriptor per contraction
chunk, issue split across the two hardware-DGE engines (SP+ACT).
Attention runs in 2 q-blocks of 1024 x 4 heads, ACT(exp)-paced; scores
stay in [k, q] orientation and the PV product streams exp-scores against
a stationary V slice plus a ones column, giving [dk+1, q] with the
softmax denominator in row dk.  A rank-1 PE matmul broadcasts the
denominator row (keeping gpsimd free for collectives) -> DVE fast
reciprocal -> fused multiply.  Scores of block i+1 interleave with the
PV matmuls of block i.

Collectives cost ~15-20us nearly independent of size, so the gather
schedule packs them: q-block 0 ships as one 4-head AllGather mid-
attention; q-block 1 ships heads 0/1 individually while the CC stream is
idle and heads 2+3 as a single pair-gather - the only collective on the
critical path.  The column-split output projections run after attention
(ACT quiet, PE unthrottled); the two q-block-1 projections accumulate
their early-gathered features first so half of each tail GEMM overlaps
the final pair-gather.
"""

import numpy as np
import ml_dtypes

import concourse.bass as bass
import concourse.tile as tile
from concourse import bacc, mybir
from concourse.bass_utils import run_bass_kernel_spmd

BF16 = mybir.dt.bfloat16
F32 = mybir.dt.float32
NPBF16 = ml_dtypes.bfloat16

B, S, D, H = 2, 2048, 1024, 16
DK = 64
DK1 = DK + 1
N_CORES = 8
HPC = 4               # heads per core
FEAT = HPC * DK       # 256 projected features per core
VW = HPC * DK1        # 260: v with a ones column per head
OCOL = 256            # output columns per core
TOKC = 1024           # token chunk for projections
QCH = 512             # q chunk for attention (= AllGather granularity)
NCH = S // QCH        # 4 chunks
NKT = S // 128        # 16 k tiles
NKC = D // 128        # 8 contraction chunks

_CACHE = {}


def _build_program():
    if "nc" in _CACHE:
        return _CACHE["nc"]

    nc = bacc.Bacc("TRN2", target_bir_lowering=False, debug=False,
                   num_devices=N_CORES)

    xq = nc.declare_dram_parameter("xq", [D, S], BF16, isOutput=False)
    xk = nc.declare_dram_parameter("xk", [D, S], BF16, isOutput=False)
    xv = nc.declare_dram_parameter("xv", [D, S], BF16, isOutput=False)
    wq = nc.declare_dram_parameter("wq", [D, FEAT], BF16, isOutput=False)
    wk = nc.declare_dram_parameter("wk", [D, FEAT], BF16, isOutput=False)
    wv = nc.declare_dram_parameter("wv", [D, VW], BF16, isOutput=False)
    wo = nc.declare_dram_parameter("wo", [D, OCOL], BF16, isOutput=False)
    bq = nc.declare_dram_parameter("bq", [128, 2], F32, isOutput=False)
    bk = nc.declare_dram_parameter("bk", [128, 2], F32, isOutput=False)
    bv = nc.declare_dram_parameter("bv", [1, VW], BF16, isOutput=False)
    bo = nc.declare_dram_parameter("bo", [1, OCOL], BF16, isOutput=False)
    out = nc.declare_dram_parameter("out", [S, OCOL], BF16, isOutput=True)
    dbg = {}

    with tile.TileContext(nc) as tc:
        with (
            tc.tile_pool(name="w", bufs=1) as wpool,
            tc.tile_pool(name="x", bufs=20) as xpool,
            tc.tile_pool(name="qk", bufs=1) as qkpool,
            tc.tile_pool(name="vp", bufs=1) as vpool,
            tc.tile_pool(name="sct", bufs=18) as sctpool,
            tc.tile_pool(name="nm", bufs=2) as nmpool,
            tc.tile_pool(name="cat", bufs=8) as catpool,
            tc.tile_pool(name="fo", bufs=3) as fopool,
            tc.tile_pool(name="ps_a", bufs=3, space="PSUM") as ps_a,
            tc.tile_pool(name="ps_pv", bufs=1, space="PSUM") as ps_pv,
            tc.tile_pool(name="dram", bufs=1, space="DRAM") as dram,
        ):
            _emit(nc, wpool, xpool, qkpool, vpool, sctpool, nmpool,
                  catpool, fopool, ps_a, ps_pv, dram,
                  xq, xk, xv, wq, wk, wv, wo, bq, bk, bv, bo, out, dbg)

    nc.compile()
    _CACHE["nc"] = nc
    return nc


def _emit(nc, wpool, xpool, qkpool, vpool, sctpool, nmpool, catpool,
          fopool, ps_a, ps_pv, dram,
          xq, xk, xv, wq, wk, wv, wo, bq, bk, bv, bo, out, dbg={}):
    MUL = mybir.AluOpType.mult
    EXPF = mybir.ActivationFunctionType.Exp
    IDF = mybir.ActivationFunctionType.Identity

    ones1 = wpool.tile([1, 128], BF16, tag="ones")
    nc.vector.memset(ones1[:], 1.0)

    # DMA issue engines for bulk input loads (round-robin: the Sync engine
    # alone issues descriptors at ~600ns each, which gates phase 1).  Only
    # SP and Activation are hardware-DGE engines; gpsimd DMA goes through
    # the software-DGE ring and corrupts data in this flow.
    dmae = [nc.sync, nc.scalar]
    NE = len(dmae)

    def load_x(src_t):
        # one 512 KB descriptor per contraction chunk: [128 rows, 4 KB/row]
        tiles = []
        for kc in range(NKC):
            t = xpool.tile([128, S], BF16, tag="xt")
            dmae[kc % NE].dma_start(t[:], src_t[bass.ts(kc, 128), :])
            tiles.append(t)
        return tiles

    wk_sb, xk_ts = [], []
    for kc in range(NKC):
        t = wpool.tile([128, FEAT], BF16, tag=f"wk{kc}")
        dmae[kc % NE].dma_start(t[:], wk[bass.ts(kc, 128), :])
        wk_sb.append(t)
        tx = xpool.tile([128, S], BF16, tag="xt")
        dmae[(kc + 1) % NE].dma_start(tx[:], xk[bass.ts(kc, 128), :])
        xk_ts.append(tx)
    bk_sb = wpool.tile([128, 2], F32, tag="bk")
    nc.sync.dma_start(bk_sb[:], bk[:])

    qh_sb = [qkpool.tile([128, S], BF16, tag=f"qh{m}", name=f"qh{m}")
             for m in range(2)]
    kh_sb = [qkpool.tile([128, S], BF16, tag=f"kh{m}", name=f"kh{m}")
             for m in range(2)]
    v_sb = [vpool.tile([128, VW], BF16, tag=f"v{j}", name=f"v{j}")
            for j in range(NKT)]

    def qk_group(w_sb, x_t, b_sb, dst, t0, m):
        ps = ps_a.tile([128, TOKC], F32, tag="a")
        for kc in range(NKC):
            for u in range(TOKC // 512):
                nc.tensor.matmul(
                    ps[:, bass.ts(u, 512)],
                    w_sb[kc][:, bass.ts(m, 128)],
                    x_t[kc][:, t0 * TOKC + u * 512:t0 * TOKC + (u + 1) * 512],
                    start=(kc == 0), stop=(kc == NKC - 1),
                )
        nc.vector.tensor_scalar_add(dst[m][:, bass.ts(t0, TOKC)], ps[:],
                                    b_sb[:, m:m + 1])

    # ---- K projection (scores need the full kh) ------------------
    for t0 in range(S // TOKC):
        for m in range(2):
            qk_group(wk_sb, xk_ts, bk_sb, kh_sb, t0, m)

    wq_sb = []
    for kc in range(NKC):
        t = wpool.tile([128, FEAT], BF16, tag=f"wq{kc}")
        dmae[kc % NE].dma_start(t[:], wq[bass.ts(kc, 128), :])
        wq_sb.append(t)
    bq_sb = wpool.tile([128, 2], F32, tag="bq")
    nc.sync.dma_start(bq_sb[:], bq[:])
    xq_ts = load_x(xq)
    for t0 in range(S // TOKC):
        for m in range(2):
            qk_group(wq_sb, xq_ts, bq_sb, qh_sb, t0, m)

    # ---- V weights + inputs --------------------------------------
    wv_sb = []
    for kc in range(NKC):
        t = wpool.tile([128, VW], BF16, tag=f"wv{kc}")
        dmae[kc % NE].dma_start(t[:], wv[bass.ts(kc, 128), :])
        wv_sb.append(t)
    bv_sb = wpool.tile([1, VW], BF16, tag="bv")
    nc.sync.dma_start(bv_sb[:], bv[:])
    xv_ts = load_x(xv)

    def v_group(t0, j):
        ps = ps_a.tile([128, VW], F32, tag="a")
        for kc in range(NKC):
            nc.tensor.matmul(
                ps[:],
                xv_ts[kc][:, t0 * TOKC + j * 128:t0 * TOKC + (j + 1) * 128],
                wv_sb[kc][:],
                start=(kc == 0), stop=False,
            )
        nc.tensor.matmul(ps[:], ones1[:], bv_sb[:], start=False, stop=True)
        nc.vector.tensor_copy(v_sb[t0 * (TOKC // 128) + j][:], ps[:])

    for t0 in range(S // TOKC):
        for j in range(TOKC // 128):
            v_group(t0, j)

    # wo + bo requested now: the 0.5 MB load drains during attention.
    wo_sb = []
    for kc in range(NKC):
        t = wpool.tile([128, OCOL], BF16, tag=f"wo{kc}")
        dmae[kc % NE].dma_start(t[:], wo[bass.ts(kc, 128), :])
        wo_sb.append(t)
    bo_sb = wpool.tile([1, OCOL], BF16, tag="bo")
    nc.sync.dma_start(bo_sb[:], bo[:])

    # ---- phase 2/3: attention + chunked AllGather + out proj -----
    # Attention runs in 2 q-blocks of 1024 x 4 heads (the baseline shape,
    # which paces best under the power throttle).  AllGathers fire per
    # 512-token chunk as soon as its heads are normalized; chunk 3 ships
    # in two half-gathers (heads 01 early, heads 23 at the end).  The
    # column-split output projections all run after attention, where the
    # ACT engine is quiet and the PE runs unthrottled; their collectives
    # are complete by then, so there is no dead zone.
    QB = 2 * QCH          # 1024-token attention block
    NQB = S // QB         # 2 blocks
    # Collectives cost ~15-20us nearly independent of size, so pack them
    # to keep the CC stream clear for the final gather: q-block 0 ships as
    # ONE 4-head gather; q-block 1 ships heads 0 and 1 individually (the
    # stream is idle then) and heads 2+3 as a single pair-gather, which is
    # the only collective left on the critical path.
    agm_in = dram.tile([FEAT, QB], BF16, tag="agmi", name="agmi")
    agm_out = dram.tile([4 * FEAT, QB], BF16, tag="agmo", name="agmo")
    agh_in = [dram.tile([DK, QB], BF16, tag=f"aghi{h}", name=f"aghi{h}")
              for h in range(2)]
    agh_out = [dram.tile([4 * DK, QB], BF16, tag=f"agho{h}",
                         name=f"agho{h}") for h in range(2)]
    agp_in = dram.tile([2 * DK, QB], BF16, tag="agpi", name="agpi")
    agp_out = dram.tile([4 * 2 * DK, QB], BF16, tag="agpo", name="agpo")

    def emit_ag(eng, ins_ap, outs_ap):
        eng.collective_compute(
            "AllGather", mybir.AluOpType.bypass,
            replica_groups=[[0, 1, 2, 3], [4, 5, 6, 7]],
            ins=[ins_ap.opt()],
            outs=[outs_ap.opt()],
        )

    def norm_and_out(pv, h, qb):
        pvs = nmpool.tile([DK1, QB], F32, tag="pvs")
        nc.vector.tensor_copy(pvs[:], pv[:])
        drow = nmpool.tile([1, QB], BF16, tag="drow")
        nc.vector.tensor_copy(drow[:], pvs[DK:DK1, :])
        # Broadcast the denominator row via a rank-1 PE matmul instead of
        # gpsimd partition_broadcast, keeping the gpsimd queue free for the
        # AllGathers (its queue blocks while a collective is in flight).
        # NOTE: reciprocal_approx_fast must read at partition base 0 (it
        # silently corrupts at non-zero bases), so it runs on the
        # broadcast [64, QB] tile, not the row.
        psb = ps_a.tile([DK, QB], F32, tag="a", name="psb")
        for u in range(QB // 512):
            nc.tensor.matmul(psb[:, bass.ts(u, 512)], ones1[0:1, 0:DK],
                             drow[:, bass.ts(u, 512)], start=True, stop=True)
        dbs = nmpool.tile([DK, QB], F32, tag="db")
        nc.vector.tensor_copy(dbs[:], psb[:])
        rb = nmpool.tile([DK, QB], F32, tag="rb")
        nc.vector.reciprocal_approx_fast(rb[:], dbs[:])
        onrm = nmpool.tile([DK, QB], BF16, tag="onrm")
        nc.vector.scalar_tensor_tensor(onrm[:], pvs[0:DK, :], 1.0, rb[:],
                                       MUL, MUL)
        if qb == 0:
            nc.sync.dma_start(agm_in[h * DK:(h + 1) * DK, :], onrm[:])
            if h == HPC - 1:
                emit_ag(nc.gpsimd, agm_in[:], agm_out[:])
        elif h < 2:
            nc.sync.dma_start(agh_in[h][:], onrm[:])
            emit_ag(nc.gpsimd, agh_in[h][:], agh_out[h][:])
        else:
            nc.sync.dma_start(agp_in[(h - 2) * DK:(h - 1) * DK, :], onrm[:])
            if h == HPC - 1:
                emit_ag(nc.gpsimd, agp_in[:], agp_out[:])

    def out_proj(c):
        qb, u = c // 2, c % 2
        # For q-block 1 the heads-0/1 features arrive (agh gathers) well
        # before the heads-2/3 pair gather: load and accumulate them first
        # so half of each tail GEMM overlaps the final collective.
        kc_order = list(range(NKC)) if qb == 0 else [0, 1, 4, 5, 2, 3, 6, 7]
        cat = {}
        for kc in kc_order:
            # features kc*128 = core r = kc//2, heads 2*(kc%2), 2*(kc%2)+1
            r, hp = kc // 2, 2 * (kc % 2)
            t = catpool.tile([128, QCH], BF16, tag="cat")
            if qb == 0:
                dmae[kc % NE].dma_start(
                    t[:], agm_out[bass.ts(kc, 128), bass.ts(u, QCH)])
            elif hp == 0:
                for hh in range(2):
                    dmae[(kc + hh) % NE].dma_start(
                        t[hh * DK:(hh + 1) * DK, :],
                        agh_out[hh][bass.ts(r, DK), bass.ts(u, QCH)])
            else:
                dmae[kc % NE].dma_start(
                    t[:], agp_out[bass.ts(r, 128), bass.ts(u, QCH)])
            cat[kc] = t
        for qt in range(QCH // 128):
            ps = ps_a.tile([128, OCOL], F32, tag="a", name="po")
            nc.tensor.matmul(ps[:], ones1[:, 0:128], bo_sb[:],
                             start=True, stop=False)
            for i, kc in enumerate(kc_order):
                nc.tensor.matmul(
                    ps[:],
                    cat[kc][:, bass.ts(qt, 128)],
                    wo_sb[kc][:],
                    start=False, stop=(i == NKC - 1),
                )
            fo = fopool.tile([128, OCOL], BF16, tag="fo")
            nc.scalar.activation(fo[:], ps[:], IDF)
            dmae[qt % NE].dma_start(
                out[bass.ts(c * (QCH // 128) + qt, 128), :], fo[:])

    blocks = [(qb, h) for qb in range(NQB) for h in range(HPC)]
    last = len(blocks) - 1
    prev = None
    for bi, (qb, h) in enumerate(blocks):
        ht, hr = h // 2, (h % 2) * 64
        q0 = qb * QB
        if bi == last:
            pv = ps_a.tile([DK1, QB], F32, tag="a", name="pv_last")
        else:
            pv = ps_pv.tile([DK1, QB], F32, tag="pv")
        cur_sc = []
        for kt in range(NKT):
            ps = ps_a.tile([128, QB], F32, tag="a")
            for u in range(QB // 512):
                nc.tensor.matmul(
                    ps[:, bass.ts(u, 512)],
                    kh_sb[ht][hr:hr + 64, bass.ts(kt, 128)],
                    qh_sb[ht][hr:hr + 64, q0 + u * 512:q0 + (u + 1) * 512],
                    start=True, stop=True,
                )
            sct = sctpool.tile([128, QB], BF16, tag="sct", name="sct")
            nc.scalar.activation(sct[:], ps[:], EXPF, scale=0.125)
            cur_sc.append(sct)
            if prev is not None:
                ppv, psc, ph, pqb = prev
                for u in range(QB // 512):
                    nc.tensor.matmul(
                        ppv[:, bass.ts(u, 512)],
                        v_sb[kt][:, ph * DK1:(ph + 1) * DK1],
                        psc[kt][:, bass.ts(u, 512)],
                        start=(kt == 0), stop=(kt == NKT - 1),
                    )
            if bi == last and kt >= 1:
                # self-interleave: the last block folds its own PV in with
                # a one-slot lag so the drain after the loop is only kt=15.
                for u in range(QB // 512):
                    nc.tensor.matmul(
                        pv[:, bass.ts(u, 512)],
                        v_sb[kt - 1][:, h * DK1:(h + 1) * DK1],
                        cur_sc[kt - 1][:, bass.ts(u, 512)],
                        start=(kt - 1 == 0), stop=False,
                    )
        if prev is not None:
            norm_and_out(prev[0], prev[2], prev[3])
        prev = (pv, cur_sc, h, qb)

    # drain: the last block only needs kt=15
    ppv, psc, ph, pqb = prev
    for u in range(QB // 512):
        nc.tensor.matmul(
            ppv[:, bass.ts(u, 512)],
            v_sb[NKT - 1][:, ph * DK1:(ph + 1) * DK1],
            psc[NKT - 1][:, bass.ts(u, 512)],
            start=False, stop=True,
        )
    norm_and_out(ppv, ph, pqb)
    for c in range(NCH):
        out_proj(c)


def _prep_inputs(q, k, v, Wq, bq, Wk, bk, Wv, bv, Wo, bo):
    """Build the per-core input maps (host-side sharding)."""
    in_maps = []
    for c in range(N_CORES):
        b, hg = c // 4, c % 4
        fsl = slice(FEAT * hg, FEAT * (hg + 1))
        osl = slice(OCOL * hg, OCOL * (hg + 1))
        wv_aug = np.zeros((D, VW), np.float32)
        bv_aug = np.zeros((VW,), np.float32)
        for h in range(HPC):
            rows = slice(FEAT * hg + DK * h, FEAT * hg + DK * (h + 1))
            wv_aug[:, h * DK1:h * DK1 + DK] = Wv[rows, :].T
            bv_aug[h * DK1:h * DK1 + DK] = bv[rows]
            bv_aug[h * DK1 + DK] = 1.0
        in_maps.append({
            "xq": np.ascontiguousarray(q[b].T).astype(NPBF16),
            "xk": np.ascontiguousarray(k[b].T).astype(NPBF16),
            "xv": np.ascontiguousarray(v[b].T).astype(NPBF16),
            "wq": np.ascontiguousarray(Wq[fsl].T).astype(NPBF16),
            "wk": np.ascontiguousarray(Wk[fsl].T).astype(NPBF16),
            "wv": wv_aug.astype(NPBF16),
            "wo": np.ascontiguousarray(Wo[osl].T).astype(NPBF16),
            "bq": np.ascontiguousarray(
                bq[fsl].reshape(2, 128).T).astype(np.float32),
            "bk": np.ascontiguousarray(
                bk[fsl].reshape(2, 128).T).astype(np.float32),
            "bv": bv_aug.reshape(1, VW).astype(NPBF16),
            "bo": np.ascontiguousarray(
                bo[osl].reshape(1, OCOL)).astype(NPBF16),
        })
    return in_maps


def run_sharded(in_maps, trace=False):
    nc = _build_program()
    res = run_bass_kernel_spmd(nc, in_maps, list(range(N_CORES)), trace=trace)
    full = np.empty((B, S, D), np.float32)
    for c in range(N_CORES):
        b, hg = c // 4, c % 4
        full[b, :, OCOL * hg:OCOL * (hg + 1)] = (
            res.results[c]["out"].astype(np.float32))
    return full, res


def kernel(q, k, v, Wq, bq, Wk, bk, Wv, bv, Wo, bo):
    args = [np.asarray(x, np.float32) for x in
            (q, k, v, Wq, bq, Wk, bk, Wv, bv, Wo, bo)]
    in_maps = _prep_inputs(*args)
    full, _ = run_sharded(in_maps)
    return full
